# revision 7
# baseline (speedup 1.0000x reference)
"""CharRNN (2-layer LSTM + softmax CE) Trainium2 Bass kernel.

Sharding: data-parallel over batch (B=64 -> 8 rows/core on 8 cores).
Each core runs the full T=128 recurrence for its 8 sequences and the
cross-entropy over its own 1024 tokens; host sums the 8 partial NLLs
(final per-token ln() + reduction also on host — 8KB/core of output).

Device design (per core):
  - Interleaved cells: cell0(step s) and cell1(step s-16) share every
    ACT/DVE instruction (cell0 rows 0:8, cell1 rows 32:40; junk rows
    8:32 are zero and flow through harmlessly).
  - All gates via tanh only (sigmoid(x) = (1+tanh(x/2))/2, with the /2
    folded into the i/o/f weight columns on the host and the cell state
    kept as c' = 2c, the hidden state as h' = 2h with all h-consuming
    weights pre-halved). One activation table set (tanh AND exp) so CE
    work interleaves with the recurrence with no table thrash.
  - PSUM z tiles hold TWO steps ("births" every other iteration): the
    per-token x-part contributions (x@W0x+b0 / h0@W1x+b1, batch-
    precomputed into SBUF) are injected at tile birth by permutation-
    matrix matmuls (host-built P tensors map 16 staged token rows to
    the PE-writable 32-granular row slots {0:8,64:72}/{32:40,96:104}),
    amortizing the inject stream over 2 steps. Per-step h-part matmuls
    accumulate (start=False) into their step's row slot.
  - Embeddings are gathered AND transposed on the host (xt input), so
    the device does no indirect DMA and no x transposes. softmax_w rows
    for the targets are host-gathered too (wtg input).
  - CE: softmax_w resident in SBUF (8MB bf16); logits [128 tok, 500
    vocab] chunks computed/exp'ed/summed at ~2 units per iteration,
    filling the PE gap left by each step's tanh/DVE tail. Per-token
    target logit via a dot with the host-gathered wtg rows.
"""

import sys

for _p in ("/opt/trn_rl_repo",):
    if _p not in sys.path:
        sys.path.insert(0, _p)

import ml_dtypes
import numpy as np

import concourse.bass as bass
import concourse.mybir as mybir
import concourse.tile as tile
from concourse.bass_utils import run_bass_kernel_spmd
from concourse.masks import make_identity

F32 = mybir.dt.float32
BF16 = mybir.dt.bfloat16
I32 = mybir.dt.int32
AF = mybir.ActivationFunctionType
ALU = mybir.AluOpType
AX = mybir.AxisListType

# Problem shapes (hardcoded per contest rules).
V, B, T, U = 16000, 64, 128, 256
NCORES = 8
BL = B // NCORES            # 8 batch rows per core
NTOK = BL * T               # 1024 tokens per core
G4 = 4 * U                  # 1024 gate width
NSLOT = 8                   # xw slots of 128 tokens (16 steps) each
VC = 500                    # vocab chunk for CE
NVC = V // VC               # 32 chunks
NTT = NTOK // 128           # 8 token tiles for CE

# Gate permutation: reference z columns are [i|j|f|o]; we reorder to
# [i|o|f|j]. Slices in permuted space (all of i,o,f get tanh(x/2) with
# the 1/2 folded into the weights; j gets plain tanh):
_TI = slice(0 * U, 1 * U)
_TO = slice(1 * U, 2 * U)
_TF = slice(2 * U, 3 * U)
_TJ = slice(3 * U, 4 * U)
GPERM = np.r_[0:U, 3 * U:4 * U, 2 * U:3 * U, U:2 * U]

FORGET_BIAS = 1.0
LAG = 16                    # cell1 lags cell0 (even; multiple of 8)

_CACHE = {}


def _split_multiwaits(nc):
    """Walrus codegen supports only ONE semaphore wait per HW instruction
    (NEURON_ISA_TPB_EVENTS has a single wait slot) and errors out on
    instructions carrying more. Tile emits multi-wait sync_info freely, so
    split: for an instruction with k>1 waits, insert k-1 NoOps on the same
    engine queue immediately before it, each carrying one wait. Identical
    semantics (the queue processes waits in order)."""
    cnt = 0
    for fn in nc.m.functions:
        for b in fn.blocks:
            live = b.instructions
            out = []
            changed = False
            for i in live:
                si = getattr(i, "sync_info", None)
                waits = list(si.on_wait) if si is not None else []
                movable_idx = [
                    k for k, w in enumerate(waits)
                    if getattr(w, "wait_reg", None) is None
                ]
                if len(waits) > 1 and len(movable_idx) >= 1:
                    hoist = movable_idx[:-1] if len(movable_idx) == len(waits) \
                        else movable_idx
                    hoist_set = set(hoist)
                    if len(waits) - len(hoist_set) > 1:
                        hoist_set = set(movable_idx)
                    for k in sorted(hoist_set):
                        nop = mybir.InstNoOp(name=f"I-nopw{cnt}", ins=[], outs=[])
                        cnt += 1
                        nop.engine = i.engine
                        nop.sync_info = mybir.SyncInfo(
                            on_wait=[waits[k]], on_update=[])
                        out.append(nop)
                    keep = [w for k, w in enumerate(waits) if k not in hoist_set]
                    i.sync_info = mybir.SyncInfo(
                        on_wait=keep, on_update=list(si.on_update))
                    changed = True
                out.append(i)
            if changed:
                live.clear()
                live.extend(out)
    return cnt


def _build_program(b0_nonzero: bool, b1_nonzero: bool, smb_nonzero: bool):
    nc = bass.Bass()

    w0 = nc.declare_dram_parameter("w0", [2 * U, G4], BF16, isOutput=False)
    w1 = nc.declare_dram_parameter("w1", [2 * U, G4], BF16, isOutput=False)
    b0v = nc.declare_dram_parameter("b0v", [1, G4], BF16, isOutput=False)
    b1v = nc.declare_dram_parameter("b1v", [1, G4], BF16, isOutput=False)
    smw = nc.declare_dram_parameter("smw", [U, V], BF16, isOutput=False)
    smb = nc.declare_dram_parameter("smb", [1, V], BF16, isOutput=False)
    xt_in = nc.declare_dram_parameter("xt_in", [128, NSLOT, 2, 128], BF16,
                                      isOutput=False)
    perm_in = nc.declare_dram_parameter("perm_in", [128, 16, 128], BF16,
                                        isOutput=False)
    wtg_in = nc.declare_dram_parameter("wtg_in", [128, NTT, 264], F32,
                                       isOutput=False)
    s_out = nc.declare_dram_parameter("s_out", [128, NTT], F32, isOutput=True)
    d_out = nc.declare_dram_parameter("d_out", [128, NTT], F32, isOutput=True)

    with tile.TileContext(nc) as tc:
        with (
            tc.tile_pool(name="const", bufs=1) as cpool,
            tc.tile_pool(name="wpool", bufs=1) as wpool,
            tc.tile_pool(name="persist", bufs=1) as ppool,
            tc.tile_pool(name="zpsum", bufs=2, space=bass.MemorySpace.PSUM) as zpool,
            tc.tile_pool(name="ptp", bufs=2, space=bass.MemorySpace.PSUM) as ptpool,
            tc.tile_pool(name="lpp", bufs=2, space=bass.MemorySpace.PSUM) as lppool,
        ):
            ident = cpool.tile([128, 128], BF16)
            make_identity(nc, ident[:])
            ones_row = cpool.tile([1, 128], BF16)
            nc.gpsimd.memset(ones_row[:], 1.0)

            # Weights resident in SBUF as 4 k-tiles each.
            w0t = wpool.tile([128, 4, G4], BF16)
            w1t = wpool.tile([128, 4, G4], BF16)
            b0t = cpool.tile([1, G4], BF16)
            b1t = cpool.tile([1, G4], BF16)
            xt = wpool.tile([128, NSLOT, 2, 128], BF16)
            perm = wpool.tile([128, 16, 128], BF16)
            wtg = wpool.tile([128, NTT, 264], F32)
            for k in range(2):
                nc.sync.dma_start(w0t[:, k, :], w0[k * 128:(k + 1) * 128, :])
            nc.sync.dma_start(b0t[:], b0v[:])
            nc.sync.dma_start(xt[:], xt_in[:])
            nc.sync.dma_start(perm[:], perm_in[:])
            for k in range(2, 4):
                nc.sync.dma_start(w0t[:, k, :], w0[k * 128:(k + 1) * 128, :])
            for k in range(4):
                nc.sync.dma_start(w1t[:, k, :], w1[k * 128:(k + 1) * 128, :])
            nc.sync.dma_start(b1t[:], b1v[:])
            if smb_nonzero:
                smbt = cpool.tile([1, V], BF16)
                nc.sync.dma_start(smbt[:], smb[:])
            # softmax_w resident: [u-half, 16000] x 2 (big: separate queue)
            smwt = wpool.tile([128, 2, V], BF16)
            for u in range(2):
                nc.scalar.dma_start(smwt[:, u, :], smw[u * 128:(u + 1) * 128, :])
            nc.gpsimd.dma_start(wtg[:], wtg_in[:])

            # Persistent buffers
            # combined transposed-h store: [layer (0=h0,1=h1), u-half, token]
            hboth = ppool.tile([128, 2, 2, NTOK], BF16)
            xw0 = ppool.tile([128, NSLOT, G4], BF16)    # batched x@Wx0+b0
            xw1 = ppool.tile([128, NSLOT, G4], BF16)    # batched h0@Wx1+b1
            nc.gpsimd.memset(xw1[:], 0.0)   # inject reads odd half early: no NaN junk
            sums = ppool.tile([128, NTT, NVC], F32)     # CE partial expsums
            souts = ppool.tile([128, NTT], F32)
            douts = ppool.tile([128, NTT], F32)

            with (
                tc.tile_pool(name="gates", bufs=3) as gpool,
                tc.tile_pool(name="state", bufs=3) as spool,
                tc.tile_pool(name="ce", bufs=3) as cepool,
            ):
                def emit_group(mms, tp=None, first_start=True):
                    for i, (o, l, r) in enumerate(mms):
                        nc.tensor.matmul(
                            o, l, r,
                            start=(i == 0 and first_start), stop=(i == len(mms) - 1),
                            tile_position=tp, skip_group_check=True)

                # Zero junk partition rows of recycled state tiles once.
                # (z-tile junk rows are zeroed every birth by the P-injects.)
                for d in range(3):
                    hz = gpool.tile([128, U], BF16, tag="h", name="hz")
                    nc.gpsimd.memset(hz[:], 0.0)
                    tcz = gpool.tile([128, U], BF16, tag="tc", name="tcz")
                    nc.gpsimd.memset(tcz[:], 0.0)
                    gz = gpool.tile([128, G4], BF16, tag="G", name="gz")
                    nc.gpsimd.memset(gz[:], 0.0)
                    cz = spool.tile([128, U], BF16, tag="c", name="cz")
                    nc.gpsimd.memset(cz[:], 0.0)
                c_prev = cz   # zeroed: full DVE chain is correct at t==0

                # ---- batched x-part precompute (lazy, per 128-token slot) ----
                xp_done = set()

                def emit_xp(slot):
                    if slot in xp_done or slot >= NSLOT:
                        return
                    xp_done.add(slot)
                    for n in range(2):
                        ns = slice(n * 512, (n + 1) * 512)
                        xp = lppool.tile([128, 512], F32, tag="lp", name="xp")
                        mms = [
                            (xp[:], xt[:, slot, 0, :], w0t[:, 0, ns]),
                            (xp[:], xt[:, slot, 1, :], w0t[:, 1, ns]),
                        ]
                        if b0_nonzero:
                            mms.append((xp[:], ones_row[0:1, :], b0t[0:1, ns]))
                        elif n == 1:
                            # forget bias always present in f cols 512:768
                            bs = slice(512, 768)
                            mms.append((xp[:, 0:256], ones_row[0:1, :],
                                        b0t[0:1, bs]))
                        emit_group(mms)
                        nc.vector.tensor_copy(xw0[:, slot, ns], xp[:])

                # ---------------- CE machinery ----------------
                ce_units = [(tk, vc) for tk in range(NTT) for vc in range(NVC)]
                ce_done = 0
                ce_pending = []
                tails_done = [0] * NTT
                epi_done = set()

                def emit_ce_mm(tk, vc):
                    lp = lppool.tile([128, VC], F32, tag="lp", name="lp")
                    mms = [
                        (lp[:], hboth[:, 1, 0, tk * 128:(tk + 1) * 128],
                         smwt[:, 0, vc * VC:(vc + 1) * VC]),
                        (lp[:], hboth[:, 1, 1, tk * 128:(tk + 1) * 128],
                         smwt[:, 1, vc * VC:(vc + 1) * VC]),
                    ]
                    if smb_nonzero:
                        mms.append((lp[:], ones_row[0:1, :],
                                    smbt[0:1, vc * VC:(vc + 1) * VC]))
                    emit_group(mms)
                    return lp

                def emit_ce_tail(lp, tk, vc):
                    es = cepool.tile([128, VC], BF16, tag="es", name="es")
                    if vc % 2 == 0:
                        # fused sum on the scalar engine (accumulator read)
                        nc.scalar.activation(es[:], lp[:], AF.Exp,
                                             accum_out=sums[:, tk, vc:vc + 1])
                    else:
                        nc.scalar.activation(es[:], lp[:], AF.Exp)
                        nc.vector.tensor_reduce(
                            out=sums[:, tk, vc:vc + 1], in_=es[:], axis=AX.X,
                            op=ALU.add)
                    tails_done[tk] += 1

                def emit_tile_epilogue_fixed(tk):
                    # ln-sum input + target logit for one 128-token tile
                    epi_done.add(tk)
                    nc.vector.tensor_reduce(
                        out=souts[:, tk:tk + 1], in_=sums[:, tk, :], axis=AX.X,
                        op=ALU.add)
                    outb = cepool.tile([128, U], F32, tag="outb", name="outb")
                    for u in range(2):
                        ptc = ptpool.tile([128, 128], F32, tag="pt", name="ptc")
                        nc.tensor.matmul(
                            ptc[:], hboth[:, 1, u, tk * 128:(tk + 1) * 128],
                            ident[:], start=True, stop=True)
                        nc.vector.tensor_copy(outb[:, u * 128:(u + 1) * 128],
                                              ptc[:])
                    ttr = cepool.tile([128, U], F32, tag="ttr", name="ttr")
                    nc.vector.tensor_tensor(
                        out=ttr[:], in0=outb[:], in1=wtg[:, tk, 0:U],
                        op=ALU.mult)
                    dtmp = cepool.tile([128, 1], F32, tag="dtmp", name="dtmp")
                    nc.vector.tensor_reduce(
                        out=dtmp[:], in_=ttr[:], axis=AX.X, op=ALU.add)
                    nc.vector.tensor_tensor(
                        out=douts[:, tk:tk + 1], in0=dtmp[:],
                        in1=wtg[:, tk, U:U + 1], op=ALU.add)

                # ---------------- LSTM + interleaved CE ----------------
                NITER = T + LAG
                z_cur = None
                z_prevgen = None

                for s in range(NITER):
                    t0 = s            # cell0 step index
                    t1 = s - LAG      # cell1 step index
                    cell0 = t0 < T
                    cell1 = 0 <= t1 < T
                    g = s % 2

                    # ---- tile birth: P-injects of both cells' x-parts ----
                    if g == 0:
                        emit_xp(s // 16)
                        if s % 16 == 2:
                            emit_xp(s // 16 + 1)   # prefetch next slot
                        z_prevgen = z_cur
                        z_cur = zpool.tile([128, G4], F32, tag="z")
                        first = True
                        if cell0:
                            w = (s // 2) % 8
                            slot = s // 16
                            for n in range(2):
                                ns = slice(n * 512, (n + 1) * 512)
                                nc.tensor.matmul(
                                    z_cur[:, ns], perm[:, w, :],
                                    xw0[:, slot, ns],
                                    start=first, stop=True,
                                    skip_group_check=True)
                            first = False
                        if cell1:
                            m = t1 // 8
                            wp = 8 + ((m % 2) * 64 + (t1 % 8) * 8) // 16
                            for n in range(2):
                                ns = slice(n * 512, (n + 1) * 512)
                                nc.tensor.matmul(
                                    z_cur[:, ns], perm[:, wp, :],
                                    xw1[:, m // 2, ns],
                                    start=first, stop=True,
                                    skip_group_check=True)
                            first = False

                    # ---- batched h0 @ Wx1 + b1 for a finished 8-step group ----
                    if s >= 8 and s % 8 == 0 and (s // 8 - 1) < T // 8:
                        m = s // 8 - 1
                        for n in range(2):
                            ns = slice(n * 512, (n + 1) * 512)
                            zx = lppool.tile([128, 512], F32, tag="lp", name="zx")
                            mms = [
                                (zx[0:64, :], hboth[:, 0, 0, m * 64:(m + 1) * 64],
                                 w1t[:, 0, ns]),
                                (zx[0:64, :], hboth[:, 0, 1, m * 64:(m + 1) * 64],
                                 w1t[:, 1, ns]),
                            ]
                            if b1_nonzero:
                                mms.append((zx[0:64, :], ones_row[0:1, 0:64],
                                            b1t[0:1, ns]))
                            elif n == 1:
                                mms.append((zx[0:64, 0:256], ones_row[0:1, 0:64],
                                            b1t[0:1, 512:768]))
                            emit_group(mms)
                            nc.vector.tensor_copy(
                                xw1[(m % 2) * 64:(m % 2) * 64 + 64, m // 2, ns],
                                zx[0:64, :])

                    zrow = 64 * g

                    # ---- per-step recurrent h-part matmuls ----
                    if cell0 and t0 >= 1:
                        for n in range(2):
                            ns = slice(n * 512, (n + 1) * 512)
                            mms = [
                                (z_cur[zrow:zrow + BL, ns],
                                 hboth[:, 0, 0, (t0 - 1) * BL:t0 * BL],
                                 w0t[:, 2, ns]),
                                (z_cur[zrow:zrow + BL, ns],
                                 hboth[:, 0, 1, (t0 - 1) * BL:t0 * BL],
                                 w0t[:, 3, ns]),
                            ]
                            emit_group(mms, first_start=False)
                    if cell1 and t1 >= 1:
                        for n in range(2):
                            ns = slice(n * 512, (n + 1) * 512)
                            mms = [
                                (z_cur[zrow + 32:zrow + 32 + BL, ns],
                                 hboth[:, 1, 0, (t1 - 1) * BL:t1 * BL],
                                 w1t[:, 2, ns]),
                                (z_cur[zrow + 32:zrow + 32 + BL, ns],
                                 hboth[:, 1, 1, (t1 - 1) * BL:t1 * BL],
                                 w1t[:, 3, ns]),
                            ]
                            emit_group(mms, tp=(0, 32 + zrow), first_start=False)

                    # ---- interleaved CE matmuls (fill the tanh/DVE tail) ----
                    ready = NVC * max(0, min(NTT - 1, (s - 32) // 16 + 1))
                    target = min(ready, 2 * max(0, s - 31))
                    while ce_done < target:
                        tk, vc = ce_units[ce_done]
                        ce_pending.append((emit_ce_mm(tk, vc), tk, vc))
                        ce_done += 1

                    if cell0 and cell1:
                        lo, hi = 0, 40
                    elif cell0:
                        lo, hi = 0, BL
                    else:
                        lo, hi = 32, 40

                    # gates: one tanh over all 1024 cols (i,o,f pre-scaled by
                    # 1/2 in the weights; j plain); junk rows 8:32 are zero.
                    tg = gpool.tile([128, G4], BF16, tag="G", name="tg")
                    nc.scalar.activation(tg[lo:hi, :],
                                         z_cur[zrow + lo:zrow + hi, :], AF.Tanh)

                    # c' = 2c recurrence:
                    # c'_new = c' * (1+tf)/2 + (1+ti) * tj
                    m2 = spool.tile([128, U], BF16, tag="m2", name="m2")
                    nc.vector.scalar_tensor_tensor(
                        out=m2[lo:hi, :], in0=tg[lo:hi, _TI], scalar=1.0,
                        in1=tg[lo:hi, _TJ], op0=ALU.add, op1=ALU.mult)
                    s1 = spool.tile([128, U], BF16, tag="s1", name="s1")
                    nc.vector.tensor_scalar(
                        s1[lo:hi, :], tg[lo:hi, _TF], 1.0, 0.5,
                        op0=ALU.add, op1=ALU.mult)
                    m1 = spool.tile([128, U], BF16, tag="m1", name="m1")
                    nc.vector.tensor_tensor(
                        out=m1[lo:hi, :], in0=s1[lo:hi, :], in1=c_prev[lo:hi, :],
                        op=ALU.mult)
                    cp_new = spool.tile([128, U], BF16, tag="c", name="cp_new")
                    nc.vector.tensor_tensor(
                        out=cp_new[lo:hi, :], in0=m1[lo:hi, :], in1=m2[lo:hi, :],
                        op=ALU.add)

                    # h' = 2h = tanh(c'/2) * (1+to)  (h-consumers pre-halved)
                    tc_t = gpool.tile([128, U], BF16, tag="tc", name="tc_t")
                    nc.scalar.activation(tc_t[lo:hi, :], cp_new[lo:hi, :], AF.Tanh,
                                         scale=0.5)
                    h_t = gpool.tile([128, U], BF16, tag="h", name="h_t")
                    nc.vector.scalar_tensor_tensor(
                        out=h_t[lo:hi, :], in0=tg[lo:hi, _TO], scalar=1.0,
                        in1=tc_t[lo:hi, :], op0=ALU.add, op1=ALU.mult)

                    for u in range(2):
                        pt = ptpool.tile([128, 64], F32, tag="pt", name="pt")
                        nc.tensor.matmul(pt[:, 0:40], h_t[:, u * 128:(u + 1) * 128],
                                         ident[:, 0:40], start=True, stop=True)
                        if cell0 and cell1:
                            # single copy for both layers: src = pt cols
                            # {0:8, 32:40}, dst = hboth[layer, u, t*8] with a
                            # custom stride covering the (layer, token) jump
                            srcap = pt[:, 0:64].rearrange(
                                "p (a b) -> p a b", b=32)[:, :, 0:BL]
                            d0 = hboth[:, 0, u, t0 * BL:(t0 + 1) * BL]
                            dstap = bass.AP(
                                d0.tensor, d0.offset,
                                [d0.ap[0], [2 * NTOK + (t1 - t0) * BL, 2], [1, BL]])
                            nc.vector.tensor_copy(dstap, srcap)
                        elif cell0:
                            nc.vector.tensor_copy(
                                hboth[:, 0, u, t0 * BL:(t0 + 1) * BL], pt[:, 0:BL])
                        else:
                            nc.vector.tensor_copy(
                                hboth[:, 1, u, t1 * BL:(t1 + 1) * BL], pt[:, 32:40])

                    c_prev = cp_new

                    for lp_h, tk_h, vc_h in ce_pending:
                        emit_ce_tail(lp_h, tk_h, vc_h)
                    ce_pending = []
                    for tk in range(NTT):
                        if tails_done[tk] == NVC and tk not in epi_done:
                            emit_tile_epilogue_fixed(tk)

                for lp_h, tk_h, vc_h in ce_pending:
                    emit_ce_tail(lp_h, tk_h, vc_h)
                ce_pending = []
                while ce_done < NTT * NVC:
                    tk, vc = ce_units[ce_done]
                    emit_ce_tail(emit_ce_mm(tk, vc), tk, vc)
                    ce_done += 1
                for tk in range(NTT):
                    if tk not in epi_done:
                        emit_tile_epilogue_fixed(tk)

                nc.sync.dma_start(s_out[:], souts[:])
                nc.sync.dma_start(d_out[:], douts[:])

    _split_multiwaits(nc)
    return nc


def _get_program(flags):
    if flags not in _CACHE:
        _CACHE[flags] = _build_program(*flags)
    return _CACHE[flags]


def _build_perm() -> np.ndarray:
    """P tensors [128, 16, 128]: variant w<8 maps staged xw rows
    16w+b -> z rows {b | 64+b} (cell0 step pair at rows 0:8 / 64:72);
    variant 8+w maps 16w+b -> z rows {32+b | 96+b} (cell1)."""
    p = np.zeros((128, 16, 128), np.float32)
    for w in range(8):
        for b in range(8):
            p[16 * w + b, w, b] = 1.0
            p[16 * w + 8 + b, w, 64 + b] = 1.0
            p[16 * w + b, 8 + w, 32 + b] = 1.0
            p[16 * w + 8 + b, 8 + w, 96 + b] = 1.0
    return p


def _prep_host(input_data, targets, embedding, W0, b0, W1, b1, softmax_w, softmax_b):
    """Host-side layout prep: gate permutation to [i|o|f|j], the
    sigmoid-via-tanh 1/2 pre-scaling of the i/o/f columns, the h'=2h
    halving of all h-consuming weight rows, forget bias, dtype casts,
    embedding gather+transpose, and target softmax_w row gather."""
    W0p = np.ascontiguousarray(np.asarray(W0, np.float32)[:, GPERM])
    W1p = np.ascontiguousarray(np.asarray(W1, np.float32)[:, GPERM])
    b0p = np.asarray(b0, np.float32)[GPERM].copy()
    b1p = np.asarray(b1, np.float32)[GPERM].copy()
    b0_nonzero = bool(np.any(b0p))
    b1_nonzero = bool(np.any(b1p))
    smb = np.asarray(softmax_b, np.float32)
    smb_nonzero = bool(np.any(smb))

    # forget bias, then scale i/o/f (cols 0:768) by 1/2 for tanh-sigmoid
    b0e = b0p.copy()
    b0e[_TF] += FORGET_BIAS
    b1e = b1p.copy()
    b1e[_TF] += FORGET_BIAS
    W0p[:, 0:3 * U] *= 0.5
    W1p[:, 0:3 * U] *= 0.5
    b0e[0:3 * U] *= 0.5
    b1e[0:3 * U] *= 0.5

    # h' = 2h folding: halve every weight row that consumes an h
    W0p[U:2 * U, :] *= 0.5          # cell0 recurrent rows
    W1p *= 0.5                      # cell1 consumes h0 (x rows) + h1 (h rows)
    smw_s = np.asarray(softmax_w, np.float32) * 0.5

    bf = ml_dtypes.bfloat16
    emb_f = np.asarray(embedding, np.float32)
    smwT_s = np.ascontiguousarray(smw_s.T)    # [V, U]

    shared = {
        "w0": W0p.astype(bf),
        "w1": W1p.astype(bf),
        "b0v": b0e[None, :].astype(bf),
        "b1v": b1e[None, :].astype(bf),
        "smw": np.ascontiguousarray(smw_s).astype(bf),
        "smb": smb[None, :].astype(bf),
        "perm_in": _build_perm().astype(bf),
    }
    in_maps = []
    ids = np.asarray(input_data, np.int32)
    tgs = np.asarray(targets, np.int32)
    for c in range(NCORES):
        tok_e = ids[c * BL:(c + 1) * BL, :].T.reshape(-1)   # t-major [1024]
        tok_t = tgs[c * BL:(c + 1) * BL, :].T.reshape(-1)
        # xt: [hidden 128, slot 8, u 2, token 128]
        xg = emb_f[tok_e].astype(bf)                 # [1024, 256]
        xtc = np.ascontiguousarray(
            xg.reshape(NSLOT, 128, 2, 128).transpose(3, 0, 2, 1))
        # wtg: [token-in-tile 128, tile 8, 264]
        wt = np.zeros((NTOK, 264), np.float32)
        wt[:, 0:U] = smwT_s[tok_t]
        wt[:, U] = smb[tok_t]
        wtc = np.ascontiguousarray(
            wt.reshape(NTT, 128, 264).transpose(1, 0, 2))
        m = dict(shared)
        m["xt_in"] = xtc
        m["wtg_in"] = wtc
        in_maps.append(m)
    return (b0_nonzero, b1_nonzero, smb_nonzero), in_maps


def run(trace=False, **inputs):
    flags, in_maps = _prep_host(**inputs)
    nc = _get_program(flags)
    res = run_bass_kernel_spmd(nc, in_maps, list(range(NCORES)), trace=trace)
    total = 0.0
    for r in res.results:
        s = r["s_out"].astype(np.float64)
        dd = r["d_out"].astype(np.float64)
        total += float(np.sum(np.log(s) - dd))
    cost = np.float32(total / (B * T))
    return cost, res


def kernel(**inputs):
    cost, _ = run(trace=False, **inputs)
    return cost
